# revision 8
# baseline (speedup 1.0000x reference)
"""Trainium2 Bass kernel for nn_Catting: out = swapaxes(x, -1, -2).reshape(B, C, N*S).

x: [B=16, C=64, S=64, N=512] f32.  Pure data movement (transpose of the last
two axes).  Sharded data-parallel over B across 8 NeuronCores (2 batches per
core).  Per core, each of the 128 [S=64, N=512] matrices is transposed on-chip:

  - matrices are processed in stacked pairs: SBUF tile [128p=(m,s), 512]
  - 4 PE transposes per pair with a stride-4 column access pattern, so PSUM
    partition p ends up holding output columns n = 4p+t  ->  the store DMA
    sees 1KB-contiguous HBM runs
  - DVE copies PSUM->SBUF reordering (t,m,s)->(m,t,s)
  - load/store DMAs are batched 1MB per instruction (8 matrices)
"""
import sys

if "/opt/trn_rl_repo" not in sys.path:
    sys.path.insert(0, "/opt/trn_rl_repo")

import numpy as np
from contextlib import ExitStack

from concourse import bass, bacc, bass_utils, tile, masks
import concourse.mybir as mybir

F32 = mybir.dt.float32

N_CORES = 8
B, C, S, N = 16, 64, 64, 512
B_PER = B // N_CORES          # 2 batches per core
MATS = B_PER * C              # 128 [64,512] matrices per core
PAIRS = MATS // 2             # 64 stacked pairs
SUPER = 4                     # pairs per DMA super-tile (4 pairs = 8 mats = 1MB)
N_SUPER = PAIRS // SUPER      # 16 super iterations

_CACHE = {}


def _build(repeat: int = 1):
    nc = bacc.Bacc("TRN2", target_bir_lowering=False, debug=False, num_devices=N_CORES)
    # x per core: [64 pairs, 128 rows=(m,s), 512 cols=n]
    x = nc.dram_tensor("x", [PAIRS, 128, N], F32, kind="ExternalInput").ap()
    # out per core: [16 supers, 8 mats, 128 p, 256=(t,s)]
    out = nc.dram_tensor("out", [N_SUPER, 2 * SUPER, 128, 256], F32, kind="ExternalOutput").ap()

    with ExitStack() as ctx:
        tc = ctx.enter_context(tile.TileContext(nc))
        const_pool = ctx.enter_context(tc.tile_pool(name="const", bufs=1))
        in_pool = ctx.enter_context(tc.tile_pool(name="in", bufs=3))
        out_pool = ctx.enter_context(tc.tile_pool(name="out", bufs=3))
        psum_pool = ctx.enter_context(tc.tile_pool(name="psum", bufs=8, space="PSUM"))

        ident = const_pool.tile([128, 128], F32)
        masks.make_identity(nc, ident[:])

        def body():
            for sup in range(N_SUPER):
                # load 4 pairs = 1MB: dram (pair' 4, part 128, n 512) -> (part, pair', n)
                tin = in_pool.tile([128, SUPER, 128, 4], F32)  # free: (pair', n_hi, t)
                nc.sync.dma_start(
                    tin[:],
                    x[sup * SUPER:(sup + 1) * SUPER].transpose([1, 0, 2]),
                )
                tout = out_pool.tile([128, 2 * SUPER, 4, 64], F32)  # (mat8=(pair',m), t, s)
                for q in range(SUPER):
                    psum_t = psum_pool.tile([128, 4, 2, 64], F32)  # one bank: (t, m, s)
                    for t in range(4):
                        # stationary = tin[:, q, :, t]: [128 part, 128 cols stride 4]
                        nc.tensor.transpose(psum_t[:, t], tin[:, q, :, t], ident[:])
                    # psum (t, m, s) -> tout[(2q+m), t, s]: dest viewed (part, t, m, s)
                    dest = tout[:, 2 * q:2 * q + 2, :, :].transpose([0, 2, 1, 3])
                    nc.vector.tensor_copy(out=dest, in_=psum_t[:])
                # store 1MB: dram (mat8 8, part 128, ts 256) -> (part, mat8, ts)
                nc.scalar.dma_start(out[sup].transpose([1, 0, 2]), tout[:])

        if repeat == 1:
            body()
        else:
            with tc.For_i(0, repeat, 1):
                body()
    nc.compile()
    return nc


def _get_nc(repeat: int = 1):
    if repeat not in _CACHE:
        _CACHE[repeat] = _build(repeat)
    return _CACHE[repeat]


def run(x: np.ndarray, trace: bool = False, repeat: int = 1, **spmd_kwargs):
    """Run on 8 cores; returns (full output, BassKernelResults)."""
    nc = _get_nc(repeat)
    x = np.ascontiguousarray(x, dtype=np.float32)
    in_maps = [
        {"x": x[i * B_PER:(i + 1) * B_PER].reshape(PAIRS, 128, N)}
        for i in range(N_CORES)
    ]
    res = bass_utils.run_bass_kernel_spmd(
        nc, in_maps, core_ids=list(range(N_CORES)), trace=trace, **spmd_kwargs
    )
    outs = [r["out"].reshape(B_PER, C, N * S) for r in res.results]
    return np.concatenate(outs, axis=0), res


def kernel(x: np.ndarray) -> np.ndarray:
    out, _ = run(x)
    return out
